# revision 15
# baseline (speedup 1.0000x reference)
"""Trainium2 Bass kernel for MLA (absorbed-weight, eval path).

Shapes: B=4, S=2048, E=2048, C=512, NHEAD=16, DHEAD=128.
Returns (u, ckv, kr) matching reference.py.

Sharding: 8 cores = (batch b, sequence half). Each core handles Sq=1024
query rows of batch b and the full St=2048 key set of batch b (ckv/kr
computed per-core, duplicated across the pair; host picks even cores').

All GEMMs run in float32r (TF32-like, full PE rate at free-dim>=256,
~1.5e-4 rel err per GEMM). Host pre-rounds weights/activations to
11-bit-mantissa RNE so DMA-loaded tensors are valid f32r inputs.

Layout strategy (per core):
- hT [E, St] host-pre-transposed; all h-contractions use it as lhsT/rhs.
- q-chain computed feature-major (pre_cqT -> t1aT -> aqT), zero transposes.
- rope applied via signed pair-swap matmul (P_sw) + cos/sin tables, in
  feature-major [d, s] layout; kr is produced directly as krT [E, St]
  (host un-transposes the returned tensor).
- scores computed transposed [St, Sq]; softmax runs as exp (no max
  subtraction; logits are O(+-10)) + ones-matmul column sums + broadcast
  reciprocal; o computed as oT [C, Sq]; u seq-major with fused rmsnorm.
"""
import numpy as np

B, S, E, C = 4, 2048, 2048, 512
NHEAD, DHEAD = 16, 128
St, Sq = S, S // 2
EK = E // 128      # 16 contraction slices over E
CK = C // 128      # 4 contraction slices over C
NST = St // 128    # 16 seq tiles (keys)
NSQ = Sq // 128    # 8 seq tiles (queries)
EPS = float(np.finfo(np.float32).eps)
SCALE = 1.0 / 12.0  # 1/sqrt(NHEAD + DHEAD)

_CACHE = {}


def _rne11(x: np.ndarray) -> np.ndarray:
    """Round f32 to 11 mantissa bits (round-nearest-even) == device f32r."""
    x = np.ascontiguousarray(x, dtype=np.float32)
    u = x.view(np.uint32)
    shift = 12
    lsb = (u >> shift) & 1
    rounded = (u + np.uint32((1 << (shift - 1)) - 1) + lsb) & np.uint32(
        ~((1 << shift) - 1) & 0xFFFFFFFF)
    return rounded.view(np.float32)


def _build_program():
    import concourse.bacc as bacc
    import concourse.mybir as mybir
    from concourse import tile
    from contextlib import ExitStack

    f32 = mybir.dt.float32
    f32r = mybir.dt.float32r
    MULT = mybir.AluOpType.mult
    ADD = mybir.AluOpType.add
    AF = mybir.ActivationFunctionType

    nc = bacc.Bacc("TRN2", target_bir_lowering=False)

    # register EPS as a const AP so activation(bias=EPS) works
    eps_t = nc.alloc_sbuf_tensor("const-eps", [128, 1], f32)
    nc.gpsimd.memset(eps_t.ap(), EPS)
    nc.const_aps.aps[(f32, EPS)] = eps_t.ap()
    nc.all_engine_barrier()

    with tile.TileContext(nc) as tc:
      with ExitStack() as top:
        dram = top.enter_context(tc.tile_pool(name="dram", bufs=1, space="DRAM"))

        def din(name, shape, dt=f32r):
            return dram.tile(shape, dt, kind="ExternalInput", name=name,
                             uniquify=False)

        hT = din("hT", [E, St])
        hTq = din("hTq", [E, Sq])
        wdkvT = din("wdkvT", [E, C])
        wdqT = din("wdqT", [E, C])
        wuqT = din("wuqT", [C, E])
        wuk = din("wuk", [E, C])
        wqrT = din("wqrT", [C, E])
        wkrT = din("wkrT", [E, E])
        wuv = din("wuv", [E, C])
        woT = din("woT", [E, E])
        g1c = din("g1c", [C, 1], f32)
        g2rep = din("g2rep", [128, C], f32)
        g3rep = din("g3rep", [128, E], f32)
        cosq = din("cosq", [128, Sq], f32)
        sinq = din("sinq", [128, Sq], f32)
        cosk = din("cosk", [128, St], f32)
        sink = din("sink", [128, St], f32)
        pswT = din("pswT", [128, 128])
        identr = din("identr", [128, 128])
        onescol = din("onescol", [128, 1])
        onesrow = din("onesrow", [1, 128])

        u_out = dram.tile([Sq, E], f32, kind="ExternalOutput", name="u_out",
                          uniquify=False)
        ckv_out = dram.tile([St, C], f32r, kind="ExternalOutput",
                            name="ckv_out", uniquify=False)
        krT_out = dram.tile([E, St], f32r, kind="ExternalOutput",
                            name="krT_out", uniquify=False)
        exp_dram = dram.tile([128, NST, Sq], f32r, name="exp_dram")
        z_dram = dram.tile([1, Sq], f32r, name="z_dram")

        # ---- persistent stores, created in reverse-death order ----
        st_qchain = ExitStack()          # aqT then cqT; die after P3
        aq_pool = st_qchain.enter_context(tc.tile_pool(name="P_aqT", bufs=1))
        aqT = aq_pool.tile([128, CK, Sq], f32r, name="aqT")
        cq_pool = st_qchain.enter_context(tc.tile_pool(name="P_cqT", bufs=1))
        cqT = cq_pool.tile([128, CK, Sq], f32r, name="cqT")

        st_pq = ExitStack()              # pre_cqT; dies after P1c
        pq_pool = st_pq.enter_context(tc.tile_pool(name="P_pre_cqT", bufs=1))
        pre_cqT = pq_pool.tile([128, CK, Sq], f32r, name="pre_cqT")

        # ---------------- P1a: ckv ----------------
        with ExitStack() as phw:
          with ExitStack() as ph:
            wp = ph.enter_context(tc.tile_pool(name="p1_w", bufs=1))
            wdkv_subs = []
            for q in range(4):
                wsub = wp.tile([128, 4, C], f32r, tag="wdkv", bufs=4,
                               name=f"wdkv{q}")
                nc.sync.dma_start(out=wsub[:], in_=wdkvT[q * 512:(q + 1) * 512, :]
                                  .rearrange("(k p) c -> p k c", p=128))
                wdkv_subs.append(wsub)
            g2_sb = wp.tile([128, C], f32)
            nc.sync.dma_start(out=g2_sb[:], in_=g2rep[:])

            bandp = ph.enter_context(tc.tile_pool(name="p1_band", bufs=3))
            psp = ph.enter_context(tc.tile_pool(name="p1_ps", bufs=2, space="PSUM"))
            outp = ph.enter_context(tc.tile_pool(name="p1_out", bufs=3))
            stp = ph.enter_context(tc.tile_pool(name="p1_st", bufs=4))

            for st in range(NST):
                band = bandp.tile([128, EK, 128], f32r, tag="band")
                nc.sync.dma_start(out=band[:], in_=hT[:, st * 128:(st + 1) * 128]
                                  .rearrange("(k p) s -> p k s", p=128))
                ps = psp.tile([128, C], f32, tag="ps")
                for k in range(EK):
                    nc.tensor.matmul(ps[:], band[:, k], wdkv_subs[k // 4][:, k % 4],
                                     start=(k == 0), stop=(k == EK - 1))
                sqt = outp.tile([128, C], f32, tag="sqt")
                ssq = stp.tile([128, 1], f32, tag="ssq")
                nc.scalar.activation(sqt[:], ps[:], AF.Square, accum_out=ssq[:])
                sd = stp.tile([128, 1], f32, tag="sd")
                nc.scalar.activation(sd[:], ssq[:], AF.Sqrt, bias=EPS,
                                     scale=1.0 / C)
                rs = stp.tile([128, 1], f32, tag="rs")
                nc.vector.reciprocal(rs[:], sd[:])
                ckv_t = outp.tile([128, C], f32r, tag="ckv")
                nc.vector.scalar_tensor_tensor(ckv_t[:], ps[:], rs[:], g2_sb[:],
                                               op0=MULT, op1=MULT)
                nc.sync.dma_start(out=ckv_out[st * 128:(st + 1) * 128, :],
                                  in_=ckv_t[:])

          # ---------------- P1b: pre_cqT + cqT ----------------
          with ExitStack() as ph:
            wpq = ph.enter_context(tc.tile_pool(name="p1b_w", bufs=1))
            g1_sb = wpq.tile([128, CK, 1], f32)
            nc.sync.dma_start(out=g1_sb[:], in_=g1c.rearrange(
                "(t p) o -> p t o", p=128))
            ones_r = wpq.tile([1, 128], f32r)
            nc.sync.dma_start(out=ones_r[:], in_=onesrow[:])
            ones_c = wpq.tile([128, 1], f32r)
            nc.sync.dma_start(out=ones_c[:], in_=onescol[:])
            wdq_subs, hq_subs = [], []
            for q in range(4):
                wsub = wpq.tile([128, 4, C], f32r, tag="wdq", bufs=4,
                                name=f"wdq{q}")
                nc.sync.dma_start(out=wsub[:], in_=wdqT[q * 512:(q + 1) * 512, :]
                                  .rearrange("(k p) c -> p k c", p=128))
                wdq_subs.append(wsub)
                hsub = wpq.tile([128, 4, Sq], f32r, tag="hq", bufs=4,
                                name=f"hq{q}")
                nc.sync.dma_start(out=hsub[:], in_=hTq[q * 512:(q + 1) * 512, :]
                                  .rearrange("(k p) s -> p k s", p=128))
                hq_subs.append(hsub)

            psp = ph.enter_context(tc.tile_pool(name="p1b_ps", bufs=2, space="PSUM"))
            for ct in range(CK):
                for ch in range(2):
                    ps = psp.tile([128, 512], f32, tag="ps")
                    for k in range(EK):
                        nc.tensor.matmul(
                            ps[:], wdq_subs[k // 4][:, k % 4,
                                                    ct * 128:(ct + 1) * 128],
                            hq_subs[k // 4][:, k % 4, ch * 512:(ch + 1) * 512],
                            start=(k == 0), stop=(k == EK - 1))
                    nc.scalar.copy(pre_cqT[:, ct, ch * 512:(ch + 1) * 512], ps[:])

            # rmsnorm over C (partition dim) via ones-matmul
            sqp = ph.enter_context(tc.tile_pool(name="p1b_sq", bufs=1))
            psz = ph.enter_context(tc.tile_pool(name="p1b_psz", bufs=1, space="PSUM"))
            psq = psz.tile([1, Sq], f32)
            sq_tiles = []
            for ct in range(CK):
                sqt = sqp.tile([128, Sq], f32r, tag=f"sq{ct}", name=f"sqt{ct}")
                nc.scalar.activation(sqt[:], pre_cqT[:, ct], AF.Square)
                sq_tiles.append(sqt)
            for ct in range(CK):
                for ch in range(2):
                    nc.tensor.matmul(psq[:, ch * 512:(ch + 1) * 512], ones_c[:],
                                     sq_tiles[ct][:, ch * 512:(ch + 1) * 512],
                                     start=(ct == 0), stop=(ct == CK - 1))
            rowp = ph.enter_context(tc.tile_pool(name="p1b_row", bufs=1))
            sdrow = rowp.tile([1, Sq], f32)
            nc.scalar.activation(sdrow[:], psq[:], AF.Sqrt, bias=EPS,
                                 scale=1.0 / C)
            rrow = rowp.tile([1, Sq], f32)
            nc.vector.reciprocal(rrow[:], sdrow[:])
            rrow_r = rowp.tile([1, Sq], f32r)
            nc.scalar.copy(rrow_r[:], rrow[:])
            psb = psz.tile([128, Sq], f32)
            for ch in range(2):
                nc.tensor.matmul(psb[:, ch * 512:(ch + 1) * 512], ones_r[:],
                                 rrow_r[:, ch * 512:(ch + 1) * 512],
                                 start=True, stop=True)
            rsrep = rowp.tile([128, Sq], f32)
            nc.scalar.copy(rsrep[:], psb[:])
            for ct in range(CK):
                nc.vector.scalar_tensor_tensor(
                    cqT[:, ct], pre_cqT[:, ct], g1_sb[:, ct], rsrep[:],
                    op0=MULT, op1=MULT)

        # ---------------- P1c/P1d: t1aT then aqT ----------------
        with ExitStack() as ph:
            psp = ph.enter_context(tc.tile_pool(name="p1c_ps", bufs=2,
                                                space="PSUM"))
            t1p = ph.enter_context(tc.tile_pool(name="p1c_t1", bufs=1))
            t1aT = t1p.tile([128, EK, Sq], f32r)
            with ExitStack() as ph2:
                wp2 = ph2.enter_context(tc.tile_pool(name="p1c_w", bufs=1))
                wuq_subs = []
                for q in range(CK):
                    wsub = wp2.tile([128, E], f32r, tag="wuq", bufs=CK,
                                    name=f"wuq{q}")
                    nc.sync.dma_start(out=wsub[:],
                                      in_=wuqT[q * 128:(q + 1) * 128, :])
                    wuq_subs.append(wsub)
                for et in range(EK):
                    for ch in range(2):
                        ps = psp.tile([128, 512], f32, tag="ps")
                        for k in range(CK):
                            nc.tensor.matmul(
                                ps[:], wuq_subs[k][:, et * 128:(et + 1) * 128],
                                pre_cqT[:, k, ch * 512:(ch + 1) * 512],
                                start=(k == 0), stop=(k == CK - 1))
                        nc.scalar.copy(t1aT[:, et, ch * 512:(ch + 1) * 512], ps[:])

            with ExitStack() as ph2:
                wp2 = ph2.enter_context(tc.tile_pool(name="p1d_w", bufs=1))
                wuk_subs = []
                for q in range(4):
                    wsub = wp2.tile([128, 4, C], f32r, tag="wuk", bufs=4,
                                    name=f"wuk{q}")
                    nc.sync.dma_start(out=wsub[:], in_=wuk[q * 512:(q + 1) * 512, :]
                                      .rearrange("(k p) c -> p k c", p=128))
                    wuk_subs.append(wsub)
                for ct in range(CK):
                    for ch in range(2):
                        ps = psp.tile([128, 512], f32, tag="ps")
                        for k in range(EK):
                            nc.tensor.matmul(
                                ps[:], wuk_subs[k // 4][:, k % 4,
                                                        ct * 128:(ct + 1) * 128],
                                t1aT[:, k, ch * 512:(ch + 1) * 512],
                                start=(k == 0), stop=(k == EK - 1))
                        nc.scalar.copy(aqT[:, ct, ch * 512:(ch + 1) * 512], ps[:])

        st_pq.close()  # pre_cqT dead

        # ---------------- P2: kr -> krT_out ----------------
        with ExitStack() as ph:
            cp = ph.enter_context(tc.tile_pool(name="p2_c", bufs=1))
            cosk_sb = cp.tile([128, St], f32)
            nc.sync.dma_start(out=cosk_sb[:], in_=cosk[:])
            sink_sb = cp.tile([128, St], f32)
            nc.sync.dma_start(out=sink_sb[:], in_=sink[:])
            psw_sb = cp.tile([128, 128], f32r)
            nc.sync.dma_start(out=psw_sb[:], in_=pswT[:])

            wkp = ph.enter_context(tc.tile_pool(name="p2_wk", bufs=4))
            htp = ph.enter_context(tc.tile_pool(name="p2_ht", bufs=6))
            psp = ph.enter_context(tc.tile_pool(name="p2_ps", bufs=3, space="PSUM"))
            pswp = ph.enter_context(tc.tile_pool(name="p2_psw", bufs=2, space="PSUM"))
            xp = ph.enter_context(tc.tile_pool(name="p2_x", bufs=3))
            tp = ph.enter_context(tc.tile_pool(name="p2_t", bufs=2))
            op = ph.enter_context(tc.tile_pool(name="p2_o", bufs=3))

            for hf in range(2):
                wkr_subs = []
                for q in range(4):
                    wsub = wkp.tile([128, 4, 1024], f32r, tag="wkr",
                                    name=f"wkr{hf}{q}")
                    nc.sync.dma_start(
                        out=wsub[:],
                        in_=wkrT[q * 512:(q + 1) * 512,
                                 hf * 1024:(hf + 1) * 1024].rearrange(
                            "(k p) e -> p k e", p=128))
                    wkr_subs.append(wsub)
                for chk in range(4):
                    ht_subs = []
                    for q in range(4):
                        hsub = htp.tile([128, 4, 512], f32r, tag="htw",
                                        name=f"htw{chk}{q}")
                        nc.sync.dma_start(
                            out=hsub[:],
                            in_=hT[q * 512:(q + 1) * 512,
                                   chk * 512:(chk + 1) * 512].rearrange(
                                "(k p) s -> p k s", p=128))
                        ht_subs.append(hsub)
                    for h8 in range(8):
                        h = hf * 8 + h8
                        ps = psp.tile([128, 512], f32, tag="ps")
                        for k in range(EK):
                            nc.tensor.matmul(
                                ps[:], wkr_subs[k // 4][:, k % 4,
                                                        h8 * 128:(h8 + 1) * 128],
                                ht_subs[k // 4][:, k % 4],
                                start=(k == 0), stop=(k == EK - 1))
                        xk = xp.tile([128, 512], f32r, tag="xk")
                        nc.scalar.copy(xk[:], ps[:])
                        ps_sw = pswp.tile([128, 512], f32, tag="psw")
                        nc.tensor.matmul(ps_sw[:], psw_sb[:], xk[:],
                                         start=True, stop=True)
                        csl = slice(chk * 512, (chk + 1) * 512)
                        t1 = tp.tile([128, 512], f32, tag="t1")
                        nc.vector.tensor_tensor(t1[:], xk[:], cosk_sb[:, csl],
                                                op=MULT)
                        t2 = tp.tile([128, 512], f32, tag="t2")
                        nc.vector.tensor_tensor(t2[:], ps_sw[:], sink_sb[:, csl],
                                                op=MULT)
                        krt = op.tile([128, 512], f32r, tag="krt")
                        nc.vector.tensor_tensor(krt[:], t1[:], t2[:], op=ADD)
                        nc.sync.dma_start(
                            out=krT_out[h * 128:(h + 1) * 128, csl], in_=krt[:])

        # ---------------- P2b: qr tiles (persist through P3) ----------------
        st_qr = ExitStack()
        qr_pool = st_qr.enter_context(tc.tile_pool(name="P_qr", bufs=1))
        qr_sb = qr_pool.tile([128, NHEAD, Sq], f32r, name="qr_sb")
        with ExitStack() as ph:
            cp = ph.enter_context(tc.tile_pool(name="p2b_c", bufs=1))
            cosq_sb = cp.tile([128, Sq], f32)
            nc.sync.dma_start(out=cosq_sb[:], in_=cosq[:])
            sinq_sb = cp.tile([128, Sq], f32)
            nc.sync.dma_start(out=sinq_sb[:], in_=sinq[:])
            psw_sb = cp.tile([128, 128], f32r)
            nc.sync.dma_start(out=psw_sb[:], in_=pswT[:])
            wqr_subs = []
            for q in range(CK):
                wsub = cp.tile([128, E], f32r, tag="wqr", bufs=CK,
                               name=f"wqr{q}")
                nc.sync.dma_start(out=wsub[:], in_=wqrT[q * 128:(q + 1) * 128, :])
                wqr_subs.append(wsub)

            psp = ph.enter_context(tc.tile_pool(name="p2b_ps", bufs=2, space="PSUM"))
            pswp = ph.enter_context(tc.tile_pool(name="p2b_psw", bufs=1, space="PSUM"))
            xp = ph.enter_context(tc.tile_pool(name="p2b_x", bufs=2))
            tp = ph.enter_context(tc.tile_pool(name="p2b_t", bufs=4))

            for h in range(NHEAD):
                ps = psp.tile([128, Sq], f32, tag="ps")
                for ch in range(2):
                    for k in range(CK):
                        nc.tensor.matmul(
                            ps[:, ch * 512:(ch + 1) * 512],
                            wqr_subs[k][:, h * 128:(h + 1) * 128],
                            cqT[:, k, ch * 512:(ch + 1) * 512],
                            start=(k == 0), stop=(k == CK - 1))
                xq = xp.tile([128, Sq], f32r, tag="xq")
                nc.scalar.copy(xq[:], ps[:])
                ps_sw = pswp.tile([128, Sq], f32, tag="psw")
                for ch in range(2):
                    nc.tensor.matmul(ps_sw[:, ch * 512:(ch + 1) * 512], psw_sb[:],
                                     xq[:, ch * 512:(ch + 1) * 512],
                                     start=True, stop=True)
                t1 = tp.tile([128, Sq], f32, tag="t1")
                nc.vector.tensor_tensor(t1[:], xq[:], cosq_sb[:], op=MULT)
                t2 = tp.tile([128, Sq], f32, tag="t2")
                nc.vector.tensor_tensor(t2[:], ps_sw[:], sinq_sb[:], op=MULT)
                nc.vector.tensor_tensor(qr_sb[:, h], t1[:], t2[:], op=ADD)

        # ---------------- P3: scoresT -> exp -> Z (exp/Z to DRAM) ----------------
        with ExitStack() as ph:
            cp = ph.enter_context(tc.tile_pool(name="p3_c", bufs=1))
            ident = cp.tile([128, 128], f32r)
            nc.sync.dma_start(out=ident[:], in_=identr[:])
            ones_c = cp.tile([128, 1], f32r)
            nc.sync.dma_start(out=ones_c[:], in_=onescol[:])

            krp = ph.enter_context(tc.tile_pool(name="p3_kr", bufs=2))
            ckp = ph.enter_context(tc.tile_pool(name="p3_ck", bufs=2))
            ctp = ph.enter_context(tc.tile_pool(name="p3_ct", bufs=2))
            pst = ph.enter_context(tc.tile_pool(name="p3_pst", bufs=2, space="PSUM"))
            pss = ph.enter_context(tc.tile_pool(name="p3_pss", bufs=2, space="PSUM"))
            expp = ph.enter_context(tc.tile_pool(name="p3_exp", bufs=3))
            psz = ph.enter_context(tc.tile_pool(name="p3_psz", bufs=1, space="PSUM"))
            ps_z = psz.tile([1, Sq], f32)

            for st in range(NST):
                krsl = krp.tile([128, NHEAD, 128], f32r, tag="krsl")
                nc.sync.dma_start(
                    out=krsl[:], in_=krT_out[:, st * 128:(st + 1) * 128]
                    .rearrange("(h p) s -> p h s", p=128))
                ckvblk = ckp.tile([128, C], f32r, tag="ckvblk")
                nc.sync.dma_start(out=ckvblk[:],
                                  in_=ckv_out[st * 128:(st + 1) * 128, :])
                ckvT_sl = ctp.tile([128, CK, 128], f32r, tag="ckvT")
                for ck in range(CK):
                    pt = pst.tile([128, 128], f32r, tag="pt")
                    nc.tensor.transpose(pt[:], ckvblk[:, ck * 128:(ck + 1) * 128],
                                        ident[:])
                    nc.scalar.copy(ckvT_sl[:, ck], pt[:])
                ps_s = pss.tile([128, Sq], f32, tag="ps_s")
                for ch in range(2):
                    sl = slice(ch * 512, (ch + 1) * 512)
                    for h in range(NHEAD):
                        nc.tensor.matmul(ps_s[:, sl], krsl[:, h],
                                         qr_sb[:, h, sl],
                                         start=(h == 0), stop=False)
                    for ck in range(CK):
                        nc.tensor.matmul(ps_s[:, sl], ckvT_sl[:, ck],
                                         aqT[:, ck, sl],
                                         start=False, stop=(ck == CK - 1))
                expt = expp.tile([128, Sq], f32r, tag="expt")
                nc.scalar.activation(expt[:], ps_s[:], AF.Exp, scale=SCALE)
                nc.sync.dma_start(out=exp_dram[:, st, :], in_=expt[:])
                for ch in range(2):
                    nc.tensor.matmul(ps_z[:, ch * 512:(ch + 1) * 512], ones_c[:],
                                     expt[:, ch * 512:(ch + 1) * 512],
                                     start=(st == 0), stop=(st == NST - 1))

            zp = ph.enter_context(tc.tile_pool(name="p3_z", bufs=1))
            zrow = zp.tile([1, Sq], f32)
            nc.scalar.copy(zrow[:], ps_z[:])
            rrow = zp.tile([1, Sq], f32)
            nc.vector.reciprocal(rrow[:], zrow[:])
            rrow_r = zp.tile([1, Sq], f32r)
            nc.scalar.copy(rrow_r[:], rrow[:])
            nc.sync.dma_start(out=z_dram[:], in_=rrow_r[:])

        st_qr.close()
        st_qchain.close()  # cqT, aqT dead

        # ---------------- P3b: oT = normalized (attn @ ckv).T ----------------
        st_o = ExitStack()
        o_pool = st_o.enter_context(tc.tile_pool(name="P_oT", bufs=1))
        oT = o_pool.tile([128, CK, Sq], f32r, name="oT")
        with ExitStack() as ph:
            cp = ph.enter_context(tc.tile_pool(name="p3b_c", bufs=1))
            ones_r = cp.tile([1, 128], f32r)
            nc.sync.dma_start(out=ones_r[:], in_=onesrow[:])
            rz_r = cp.tile([1, Sq], f32r)
            nc.sync.dma_start(out=rz_r[:], in_=z_dram[:])
            with ExitStack() as phb:
                psb2 = phb.enter_context(tc.tile_pool(name="p3b_psb", bufs=1,
                                                      space="PSUM"))
                ps_b = psb2.tile([128, Sq], f32)
                for ch in range(2):
                    nc.tensor.matmul(ps_b[:, ch * 512:(ch + 1) * 512], ones_r[:],
                                     rz_r[:, ch * 512:(ch + 1) * 512],
                                     start=True, stop=True)
                rzrep = cp.tile([128, Sq], f32)
                nc.scalar.copy(rzrep[:], ps_b[:])

            ckp2 = ph.enter_context(tc.tile_pool(name="p3b_ck", bufs=3))
            expp2 = ph.enter_context(tc.tile_pool(name="p3b_exp", bufs=3))
            pso = ph.enter_context(tc.tile_pool(name="p3b_ps", bufs=1,
                                                space="PSUM"))
            ps_list = [pso.tile([128, Sq], f32, tag=f"o{ct}", name=f"ps_o{ct}")
                       for ct in range(CK)]
            for stk in range(NST):
                ckvblk = ckp2.tile([128, C], f32r, tag="ckvblk")
                nc.sync.dma_start(out=ckvblk[:],
                                  in_=ckv_out[stk * 128:(stk + 1) * 128, :])
                expt = expp2.tile([128, Sq], f32r, tag="expt")
                nc.sync.dma_start(out=expt[:], in_=exp_dram[:, stk, :])
                for ct in range(CK):
                    for ch in range(2):
                        nc.tensor.matmul(
                            ps_list[ct][:, ch * 512:(ch + 1) * 512],
                            ckvblk[:, ct * 128:(ct + 1) * 128],
                            expt[:, ch * 512:(ch + 1) * 512],
                            start=(stk == 0), stop=(stk == NST - 1),
                            skip_group_check=True)
            for ct in range(CK):
                nc.vector.tensor_tensor(oT[:, ct], ps_list[ct][:], rzrep[:],
                                        op=MULT)

        # ---------------- P4: absorbed_w_o + u (fused row norm) ----------------
        with ExitStack() as ph:
            wp = ph.enter_context(tc.tile_pool(name="p4_w", bufs=1))
            wuv_subs = []
            for q in range(4):
                wsub = wp.tile([128, 4, C], f32r, tag="wuv", bufs=4,
                               name=f"wuv{q}")
                nc.sync.dma_start(out=wsub[:], in_=wuv[q * 512:(q + 1) * 512, :]
                                  .rearrange("(k p) c -> p k c", p=128))
                wuv_subs.append(wsub)
            g3_sb = wp.tile([128, E], f32)
            nc.sync.dma_start(out=g3_sb[:], in_=g3rep[:])
            absf = ph.enter_context(tc.tile_pool(name="p4_absf", bufs=1))
            abs_full = absf.tile([128, CK, E], f32r)

            with ExitStack() as pha:
                wop = ph.enter_context(tc.tile_pool(name="p4_wo", bufs=6))
                psa = pha.enter_context(tc.tile_pool(name="p4_psa", bufs=2,
                                                     space="PSUM"))
                for ech in range(4):
                    wo_subs = []
                    for q in range(4):
                        wsub = wop.tile([128, 4, 512], f32r, tag="wo",
                                        name=f"wo{ech}{q}")
                        nc.sync.dma_start(
                            out=wsub[:], in_=woT[q * 512:(q + 1) * 512,
                                                 ech * 512:(ech + 1) * 512]
                            .rearrange("(k p) e -> p k e", p=128))
                        wo_subs.append(wsub)
                    for ct in range(CK):
                        ps = psa.tile([128, 512], f32, tag="psa")
                        for k in range(EK):
                            nc.tensor.matmul(ps[:],
                                             wuv_subs[k // 4][:, k % 4,
                                                              ct * 128:(ct + 1) * 128],
                                             wo_subs[k // 4][:, k % 4],
                                             start=(k == 0), stop=(k == EK - 1))
                        nc.scalar.copy(abs_full[:, ct, ech * 512:(ech + 1) * 512],
                                       ps[:])

            psu = ph.enter_context(tc.tile_pool(name="p4_psu", bufs=2, space="PSUM"))
            outp = ph.enter_context(tc.tile_pool(name="p4_out", bufs=2))
            stp = ph.enter_context(tc.tile_pool(name="p4_st", bufs=4))
            for sq in range(NSQ):
                ps = psu.tile([128, E], f32, tag="psu")
                for ech in range(4):
                    for k in range(CK):
                        nc.tensor.matmul(ps[:, ech * 512:(ech + 1) * 512],
                                         oT[:, k, sq * 128:(sq + 1) * 128],
                                         abs_full[:, k, ech * 512:(ech + 1) * 512],
                                         start=(k == 0), stop=(k == CK - 1))
                u_t = outp.tile([128, E], f32, tag="ut", name="u_t")
                ssq = stp.tile([128, 1], f32, tag="ssq")
                nc.scalar.activation(u_t[:], ps[:], AF.Square, accum_out=ssq[:])
                sd = stp.tile([128, 1], f32, tag="sd")
                nc.scalar.activation(sd[:], ssq[:], AF.Sqrt, bias=EPS,
                                     scale=1.0 / E)
                rs = stp.tile([128, 1], f32, tag="rs")
                nc.vector.reciprocal(rs[:], sd[:])
                nc.vector.scalar_tensor_tensor(u_t[:], ps[:], rs[:],
                                               g3_sb[:], op0=MULT, op1=MULT)
                nc.sync.dma_start(out=u_out[sq * 128:(sq + 1) * 128, :],
                                  in_=u_t[:])

        st_o.close()

    nc.compile()
    return nc


def _host_inputs(h, w_d_kv, w_u_k, w_u_v, w_d_q, w_u_q, w_kr, w_qr, w_o,
                 g1, g2, g3):
    """Build the 8 per-core input maps."""
    # rope tables, computed exactly as reference (float32 throughout)
    freqs = (1.0 / (10000.0 ** (np.arange(0, DHEAD, 2, dtype=np.float32)
                                / np.float32(DHEAD)))).astype(np.float32)
    ang = np.arange(S, dtype=np.float32)[:, None] * freqs[None, :]   # [S, 64]
    cos_t = np.cos(ang).astype(np.float32)   # [S, 64]
    sin_t = np.sin(ang).astype(np.float32)
    # feature-major tables [d, s]: row d uses freq d//2
    cos_d = np.repeat(cos_t.T, 2, axis=0).reshape(DHEAD, S)  # wrong order fix below
    # np.repeat on axis 0 of [64, S] -> [128, S] with rows (f0,f0,f1,f1,...)
    cos_d = np.ascontiguousarray(np.repeat(cos_t.T, 2, axis=0))  # [128, S]
    sin_d = np.ascontiguousarray(np.repeat(sin_t.T, 2, axis=0))

    pswT = np.zeros((128, 128), dtype=np.float32)
    for i in range(64):
        pswT[2 * i + 1, 2 * i] = -1.0   # sw[2i]   = -x[2i+1]
        pswT[2 * i, 2 * i + 1] = 1.0    # sw[2i+1] = +x[2i]

    weights = {
        "wdkvT": _rne11(w_d_kv.T), "wdqT": _rne11(w_d_q.T),
        "wuqT": _rne11(w_u_q.T), "wuk": _rne11(w_u_k),
        "wqrT": _rne11(w_qr.T), "wkrT": _rne11(w_kr.T),
        "wuv": _rne11(w_u_v), "woT": _rne11(w_o.T),
        "g1c": np.ascontiguousarray(g1.reshape(C, 1), dtype=np.float32),
        "g2rep": np.ascontiguousarray(np.broadcast_to(g2, (128, C)),
                                      dtype=np.float32),
        "g3rep": np.ascontiguousarray(np.broadcast_to(g3, (128, E)),
                                      dtype=np.float32),
        "cosk": cos_d, "sink": sin_d,
        "pswT": pswT,
        "identr": np.eye(128, dtype=np.float32),
        "onescol": np.ones((128, 1), dtype=np.float32),
        "onesrow": np.ones((1, 128), dtype=np.float32),
    }

    in_maps = []
    hT_b = [_rne11(np.ascontiguousarray(h[b].T)) for b in range(B)]
    for c in range(8):
        b, half = c // 2, c % 2
        m = dict(weights)
        m["hT"] = hT_b[b]
        m["hTq"] = np.ascontiguousarray(hT_b[b][:, half * Sq:(half + 1) * Sq])
        m["cosq"] = np.ascontiguousarray(cos_d[:, half * Sq:(half + 1) * Sq])
        m["sinq"] = np.ascontiguousarray(sin_d[:, half * Sq:(half + 1) * Sq])
        in_maps.append(m)
    return in_maps


def kernel(h, w_d_kv, w_u_k, w_u_v, w_d_q, w_u_q, w_kr, w_qr, w_o, g1, g2, g3):
    from concourse.bass_utils import run_bass_kernel_spmd

    h = np.asarray(h, dtype=np.float32)
    if "nc" not in _CACHE:
        _CACHE["nc"] = _build_program()
    nc = _CACHE["nc"]

    in_maps = _host_inputs(h, np.asarray(w_d_kv), np.asarray(w_u_k),
                           np.asarray(w_u_v), np.asarray(w_d_q),
                           np.asarray(w_u_q), np.asarray(w_kr),
                           np.asarray(w_qr), np.asarray(w_o),
                           np.asarray(g1), np.asarray(g2), np.asarray(g3))
    res = run_bass_kernel_spmd(nc, in_maps, list(range(8)))
    outs = res.results

    u = np.empty((B, S, E), dtype=np.float32)
    ckv = np.empty((B, S, C), dtype=np.float32)
    kr = np.empty((B, S, E), dtype=np.float32)
    for c in range(8):
        b, half = c // 2, c % 2
        u[b, half * Sq:(half + 1) * Sq] = outs[c]["u_out"]
    for b in range(B):
        ckv[b] = outs[2 * b]["ckv_out"]
        kr[b] = np.ascontiguousarray(outs[2 * b]["krT_out"].T)
    return (u, ckv, kr)
